# revision 45
# baseline (speedup 1.0000x reference)
"""MultiHeadAxialAttention TRN2 kernel (~474us measured; prior 492us, stub 772us).

Problem: x[4,128,128,512] -> 1x1 conv q/k/v projections -> axial attention
(column attention over H, then row attention over W, per head) -> [4,128,128,512].

Sharding: core = (batch b, head-group of 4 heads); 8 cores, zero cross-core
communication. Host pre-transposes x[b] to x^T [512, 16384]; host reassembles
the per-core outputs into the reference channel order (channel = d*8 + n) and
applies the final row-softmax division (denominator is DMA'd as channel 64).

Key structure (softwarepipeline basics as before: lookahead-2 emission in both
attention passes, clips one iteration late, rotating psum tag pools, two heads
in lockstep on PE row-tiles T0/T8, double-stored K (h-major KT / w-major KTw),
next group's projections interleaved into the current group's row pass).

492us -> 474us changes (trace-driven):
  - column-softmax normalization moved from phase C into the phase-B psum
    evacuation (reciprocal of the ride-along sum plane + one fused
    multiply), so phase C is pure transposes + copies.
  - xv2 stored h-major [w, h, d]: phase D's value matmuls get a contiguous
    moving operand. Strided moving operands measured 113ns vs 54ns per MM
    (the same 2x stride penalty shows in B-scores: 218ns/pair vs D's 81ns).
  - phase C transposes batched 8 d-planes per psum tile; evacuation is an
    (h, d)-transposed AP split ACT (h 0:72) / DVE (h 72:128) - ACT handles
    strided writes at ~1.65ns/el vs DVE ~2ns/el.
  - proj psum evacuations rebalanced: contiguous q-evac on DVE, strided
    KTw/VTh evacs on ACT (the ACT chain is the psum-free critical path).
  - output DMAs split across sync + gpsimd queues (end-of-kernel drain);
    weight loads split sync/gpsimd; x prefetch on the scalar queue.

Measured dead ends (kept behind env flags, default off): HAM warm-keeper
matmuls (KWARM/KWARM2: the PE clock gate throttles to 1.2GHz during the
low-duty score/value phases and neither periodic nor burst fillers hold it
warm - the phases' intrinsic array duty is below the MID threshold), evac
halving across engines (KSPLIT: DVE is slow on strided writes), and
interleaving the A2 transposes into phase B (batched transposes pipeline
better than interleaved ones). Also measured: pairing two h-blocks per
output DMA (1040B descriptors, half the packets) regressed 470->554us -
holding the ob tile across two d_values iterations serializes the psum
pipeline, far outweighing the ~18us DMA-drain tail it was meant to fix.
Round-robining the last group's output DMAs over all three DMA-capable
queues also left the tail unchanged: every queue fans into the same
16-engine DMA pool, and 520B packets are engine-pool rate-bound, not
queue-bound. The tail needs >=1KB per-partition descriptor runs, which
needs an output layout whose ob tiles don't outlive one d_values call.
Run-to-run variance on this kernel is ~+-8us (the HAM activity window is
free-running, shifting which phases run at 1.2 vs 2.4 GHz).
"""
import sys
import os
import math

import numpy as np
import ml_dtypes

if "/opt/trn_rl_repo" not in sys.path:
    sys.path.insert(0, "/opt/trn_rl_repo")

B, H, W, C = 4, 128, 128, 512
NH, D = 8, 64
NCORES = 8
NGROUPS = 2          # head groups per core, 2 heads each
PIX = H * W          # 16384, h-major (pix = h*128 + w)
CLIP = 1.0 - 1e-7
SCALE = 1.0 / math.sqrt(D)   # 1/8
EXP_LO = float(np.float32(math.exp(-CLIP * SCALE)))
EXP_HI = float(np.float32(math.exp(CLIP * SCALE)))

_CACHE = {}


def _build_bass():
    import concourse.bacc as bacc
    import concourse.tile as tile
    import concourse.mybir as mybir
    from concourse import masks

    F32 = mybir.dt.float32
    BF16 = mybir.dt.bfloat16
    Act = mybir.ActivationFunctionType
    Alu = mybir.AluOpType

    nc = bacc.Bacc(None, target_bir_lowering=False)

    xT_d = nc.dram_tensor("xT", [4, 128, PIX], BF16, kind="ExternalInput")
    wq_d = nc.dram_tensor("wq", [4, 128, 256], BF16, kind="ExternalInput")
    wk_d = nc.dram_tensor("wk", [4, 128, 256], BF16, kind="ExternalInput")
    wv_d = nc.dram_tensor("wv", [4, 128, 256], BF16, kind="ExternalInput")
    bq_d = nc.dram_tensor("bq", [128, 2], F32, kind="ExternalInput")
    bk_d = nc.dram_tensor("bk", [128, 2], F32, kind="ExternalInput")
    bv_d = nc.dram_tensor("bv", [128, 2], F32, kind="ExternalInput")
    # [head-pair g, hb, w, jh, h_in_block, d+1] — exactly the ob tile
    # layout with the head-pair dim inner, so each hb ships as ONE DMA
    # with 1040B contiguous per partition (520B descriptors measured
    # engine-pool packet-rate bound: ~18us drain tail at kernel end)
    out_d = nc.dram_tensor("out", [2, H // 4, W, 2, 4, D + 1], BF16,
                           kind="ExternalOutput")

    with tile.TileContext(nc) as tc:
        with (
            tc.tile_pool(name="const", bufs=1) as constp,
            tc.tile_pool(name="persist", bufs=1) as persist,
            tc.tile_pool(name="xt", bufs=4) as xtp,
            tc.tile_pool(name="ebuf", bufs=5) as ebufp,
            tc.tile_pool(name="rsbuf", bufs=2) as rsp,
            tc.tile_pool(name="obuf", bufs=4) as obufp,
            tc.tile_pool(name="ps", bufs=4, space="PSUM") as psp,
        ):
            ident_bf16 = constp.tile([128, 128], BF16, tag="id16")
            ident_f32 = constp.tile([128, 128], F32, tag="id32")
            masks.make_identity(nc, ident_bf16[:])
            masks.make_identity(nc, ident_f32[:])

            wsb = {}
            bsb = {}
            xt0 = None
            for nm, wd, bd in (("q", wq_d, bq_d), ("k", wk_d, bk_d),
                               ("v", wv_d, bv_d)):
                wt = constp.tile([128, 4, 256], BF16, tag=f"w{nm}")
                for kc in range(4):
                    # split weight loads across two queues at startup
                    eng = nc.sync if kc % 2 == 0 else nc.gpsimd
                    eng.dma_start(wt[:, kc, :], wd[kc])
                if xt0 is None:
                    # prefetch the first pixel tile right after the q
                    # weights so the first projection chain starts ~6us
                    # before the k/v weight DMAs finish
                    xt0 = xtp.tile([128, 4, 512], BF16, tag="xt", name="xt0")
                    nc.scalar.dma_start(
                        xt0[:], xT_d[:, :, 0:512].transpose([1, 0, 2]))
                bt = constp.tile([128, 2], F32, tag=f"b{nm}")
                nc.sync.dma_start(bt[:], bd[:])
                wsb[nm] = wt
                bsb[nm] = bt

            QT = persist.tile([128, PIX], BF16, tag="QT")
            KT = persist.tile([128, PIX], BF16, tag="KT")
            # second copy of K in w-major order: the column-pass score
            # matmuls get contiguous LDWEIGHTS (strided measured 2.3x slower)
            KTw = persist.tile([128, PIX], BF16, tag="KTw")

            NT = PIX // 512   # 32 pixel tiles of 512

            KREPS = int(os.environ.get("KREPS", "1"))
            KSPLIT = os.environ.get("KSPLIT", "0") != "0"
            KWARM = os.environ.get("KWARM", "0") != "0"
            KWARM2 = os.environ.get("KWARM2", "0") != "0"
            KNORM = os.environ.get("KNORM", "1") != "0"

            def warm_mm(ps_slice):
                # dense full-array N=512 matmul on const weights: keeps the
                # PE HAM activity monitor at K=8/8 through the low-duty
                # score/value phases (result is overwritten by real scores)
                nc.tensor.matmul(
                    ps_slice.rearrange("p a b -> p (a b)"),
                    wsb["q"][:, 0, 0:128],
                    wsb["k"][:, 0:2, :].rearrange("p a b -> p (a b)"),
                    start=True, stop=True)

            def strided_evac(dstv, srcv, bias):
                # [128, 128, 4] strided psum evacuation split across ACT
                # and DVE halves so the psum buffer frees ~2x sooner
                nc.scalar.activation(
                    dstv[:, 0:64, :], srcv[:, 0:64, :],
                    Act.Identity, bias=bias, scale=1.0)
                nc.vector.tensor_scalar(
                    dstv[:, 64:128, :], srcv[:, 64:128, :],
                    bias, None, Alu.add)

            def a_proj(g, tt, VTh, xt=None):
                fsl = slice(g * 128, (g + 1) * 128)
                if xt is None:
                    xt = xtp.tile([128, 4, 512], BF16, tag="xt", name="xt")
                    nc.sync.dma_start(
                        xt[:],
                        xT_d[:, :, tt * 512:(tt + 1) * 512].transpose(
                            [1, 0, 2]))
                for nm, dst in (("q", QT), ("k", KT), ("v", VTh)):
                    ps = psp.tile([128, 512], F32, tag="Q", bufs=2, name="ps")
                    for kc in range(4):
                        nc.tensor.matmul(
                            ps[:], wsb[nm][:, kc, fsl], xt[:, kc, :],
                            start=(kc == 0), stop=(kc == 3))
                    if nm == "k":
                        nc.vector.tensor_scalar(
                            dst[:, tt * 512:(tt + 1) * 512], ps[:],
                            bsb[nm][:, g:g + 1], None, Alu.add)
                        kv = KTw.rearrange("p (w h) -> p w h", w=128)[
                            :, :, tt * 4:(tt + 1) * 4]
                        src = ps[:].rearrange("p (h w) -> p w h", h=4)
                        if KSPLIT:
                            strided_evac(kv, src, bsb[nm][:, g:g + 1])
                        else:
                            nc.scalar.activation(
                                kv, src, Act.Identity,
                                bias=bsb[nm][:, g:g + 1], scale=1.0)
                    elif nm == "q":
                        # contiguous evac on DVE: frees the ACT queue for
                        # the strided KTw/VTh evacs (ACT is the proj-phase
                        # psum-free critical path)
                        nc.vector.tensor_scalar(
                            dst[:, tt * 512:(tt + 1) * 512], ps[:],
                            bsb[nm][:, g:g + 1], None, Alu.add)
                    else:
                        # VTh is w-major (pix2 = w*128 + h) so phase A2's
                        # transpose reads are contiguous
                        vv = VTh.rearrange("p (w h) -> p w h", w=128)[
                            :, :, tt * 4:(tt + 1) * 4]
                        src = ps[:].rearrange("p (h w) -> p w h", h=4)
                        if KSPLIT:
                            strided_evac(vv, src, bsb[nm][:, g:g + 1])
                        else:
                            nc.scalar.activation(
                                vv, src, Act.Identity,
                                bias=bsb[nm][:, g:g + 1], scale=1.0)

            for rep in range(KREPS):
              VTh = persist.tile([128, PIX], BF16, tag="big", name="VTh")
              for tt in range(NT):
                  a_proj(0, tt, VTh, xt=(xt0 if tt == 0 and rep == 0
                                         else None))
              for g in range(NGROUPS):
                  # ---- phase A2: V^T[f, (w h)] -> V_sb[h, w, d] ----
                  # emitted interleaved into phase B (below): the dense
                  # 128-row transposes raise B's PE duty (HAM stays warm)
                  # and fill B's dependency bubbles
                  V_sb = [persist.tile([128, W, D + 1], BF16, tag=f"Vx{j}",
                                       name=f"V{j}") for j in range(2)]
                  for jh in range(2):
                      nc.vector.memset(V_sb[jh][:, :, D], 1.0)

                  def a2_block(wb2):
                      tps = psp.tile([128, 8, 128], BF16, tag="Q", bufs=2,
                                     name="tps")
                      for j in range(8):
                          w = wb2 * 8 + j
                          nc.tensor.transpose(
                              tps[:, j, :], VTh[:, w * 128:(w + 1) * 128],
                              ident_bf16[:])
                      nc.scalar.copy(
                          V_sb[0][:, wb2 * 8:wb2 * 8 + 8, 0:D],
                          tps[:, :, 0:64])
                      nc.vector.tensor_copy(
                          V_sb[1][:, wb2 * 8:wb2 * 8 + 8, 0:D],
                          tps[:, :, 64:128])

                  # ---- phase B: column attention, heads paired ----
                  xv_sb = persist.tile([128, 2, D + 1, W], BF16, tag="big",
                                       name="xv_sb")

                  def b_scores(wb):
                      sps = psp.tile([128, 8, 128], F32, tag="P", bufs=3,
                                     name="sps")
                      if KWARM:
                          warm_mm(sps[:, 0:4, :])
                      if KWARM2:
                          # burst of dense N=512 matmuls at phase entry to
                          # flip the HAM clock gate to 8/8, then a hold
                          # warmer every other iteration
                          if wb == 0:
                              for _ in range(16):
                                  warm_mm(sps[:, 0:4, :])
                          elif wb % 2 == 0:
                              warm_mm(sps[:, 0:4, :])
                      for c in range(4):
                          w = wb * 4 + c
                          for jh in range(2):
                              hsl = slice(jh * 64, (jh + 1) * 64)
                              nc.tensor.matmul(
                                  sps[:, jh * 4 + c, :],
                                  KTw[hsl, w * 128:(w + 1) * 128],
                                  QT[hsl, w::128],
                                  start=True, stop=True)
                      ex = ebufp.tile([128, 8, 128], BF16, tag="ex",
                                      name="ex")
                      nc.scalar.activation(ex[:], sps[:], Act.Exp,
                                           scale=SCALE)
                      return ex

                  def b_clip(ex):
                      nc.vector.tensor_scalar(ex[:], ex[:], EXP_LO, EXP_HI,
                                              Alu.max, Alu.min)

                  def b_values(wb, ex):
                      xvps = psp.tile([128, 8, 128], F32, tag="P", bufs=3,
                                      name="xvps")
                      for c in range(4):
                          w = wb * 4 + c
                          for jh in range(2):
                              j = jh * 4 + c
                              nc.tensor.matmul(
                                  xvps[:, j, 0:D + 1],
                                  ex[:, j, :], V_sb[jh][:, w, :],
                                  start=True, stop=True)
                      if KNORM:
                          # fold the column-softmax normalization into the
                          # evacuation: xv_sb holds xv/sv (plane D becomes 1)
                          rs = rsp.tile([128, 8, 1], F32, tag="rsb2",
                                        bufs=2, name="rs")
                          nc.vector.reciprocal(rs[:], xvps[:, :, D:D + 1])
                          nc.vector.tensor_tensor(
                              xv_sb[:, :, :, wb * 4:wb * 4 + 4],
                              xvps[:, :, 0:D + 1].rearrange(
                                  "p (j c) d -> p j d c", j=2),
                              rs[:].rearrange(
                                  "p (j c) u -> p j u c", j=2).broadcast_to(
                                  [128, 2, D + 1, 4]),
                              Alu.mult)
                      else:
                          nc.vector.tensor_copy(
                              xv_sb[:, :, :, wb * 4:wb * 4 + 4],
                              xvps[:, :, 0:D + 1].rearrange(
                                  "p (j c) d -> p j d c", j=2))

                  for wb2 in range(16):
                      a2_block(wb2)

                  exs = {}
                  for wb in range(36):
                      if wb < 32:
                          exs[wb] = b_scores(wb)
                      if wb - 4 >= 0:
                          b_values(wb - 4, exs[wb - 4])
                          del exs[wb - 4]
                      if 0 <= wb - 1 < 32:
                          b_clip(exs[wb - 1])

                  # ---- phase C: xv [h, jh, d, w] -> xv2 ----
                  if KNORM:
                      # xv already normalized at the B evac; xv2 stored
                      # h-major [w, h, d] so phase D's value matmuls get a
                      # contiguous moving operand (strided moving measured
                      # 113ns vs 54ns per MM)
                      xv2 = [persist.tile([128, H, D + 1], BF16,
                                          tag=f"Vx{j}", name=f"xv2_{j}")
                             for j in range(2)]
                      for jh in range(2):
                          for db in range(8):
                              # 8 d-planes per psum tile; evacuate with an
                              # (h, d)-transposed AP split across ACT and
                              # DVE h-ranges (ACT takes more: it handles
                              # strided writes at ~1.65ns/el vs DVE ~2)
                              mps = psp.tile([128, 8, 128], BF16, tag="Q",
                                             bufs=2, name="mps")
                              for j in range(8):
                                  d = db * 8 + j
                                  nc.tensor.transpose(
                                      mps[:, j, :], xv_sb[:, jh, d, :],
                                      ident_bf16[:])
                              src = mps[:].rearrange("p j h -> p h j")
                              dst = xv2[jh][:, :, db * 8:db * 8 + 8]
                              nc.scalar.copy(dst[:, 0:72, :], src[:, 0:72, :])
                              nc.vector.tensor_copy(dst[:, 72:128, :],
                                                    src[:, 72:128, :])
                          nc.vector.memset(xv2[jh][:, :, D], 1.0)
                  else:
                      xv2 = [persist.tile([128, D + 1, H], BF16,
                                          tag=f"Vx{j}", name=f"xv2_{j}")
                             for j in range(2)]
                      for jh in range(2):
                          stp = psp.tile([128, 128], BF16, tag="Q", bufs=2,
                                         name="stp")
                          nc.tensor.transpose(stp[:], xv_sb[:, jh, D, :],
                                              ident_bf16[:])
                          rsvT_f = rsp.tile([128, H], F32, tag="rsf",
                                            name="rsvT_f")
                          nc.vector.reciprocal(rsvT_f[:], stp[:])
                          rsvT = rsp.tile([128, H], BF16, tag="rsb",
                                          name="rsvT")
                          nc.vector.tensor_copy(rsvT[:], rsvT_f[:])
                          for db in range(16):
                              mps = psp.tile([128, 4, 128], BF16, tag="Q",
                                             bufs=2, name="mps")
                              for j in range(4):
                                  d = db * 4 + j
                                  nc.tensor.transpose(
                                      mps[:, j, :], xv_sb[:, jh, d, :],
                                      ident_bf16[:])
                              nc.vector.tensor_tensor(
                                  xv2[jh][:, db * 4:db * 4 + 4, :], mps[:],
                                  rsvT[:].unsqueeze(1).broadcast_to(
                                      [128, 4, 128]),
                                  Alu.mult)
                          nc.vector.memset(xv2[jh][:, D, :], 1.0)

                  # ---- phase D: row attention; next group's projections
                  # are interleaved per-hb (QT/KT slice tt is last read by
                  # the scores of hb=tt, so the overwrite pipelines) ----
                  merge = (g + 1 < NGROUPS) or (rep + 1 < KREPS)
                  if merge:
                      VTh = persist.tile([128, PIX], BF16, tag="big",
                                         name="VTh")

                  def d_scores(hb):
                      sps2 = psp.tile([128, 8, 128], F32, tag="P", bufs=3,
                                      name="sps2")
                      if KWARM and not merge:
                          warm_mm(sps2[:, 0:4, :])
                      if KWARM2 and not merge:
                          if hb == 0:
                              for _ in range(16):
                                  warm_mm(sps2[:, 0:4, :])
                          elif hb % 2 == 0:
                              warm_mm(sps2[:, 0:4, :])
                      for c in range(4):
                          h = hb * 4 + c
                          for jh in range(2):
                              hsl = slice(jh * 64, (jh + 1) * 64)
                              nc.tensor.matmul(
                                  sps2[:, jh * 4 + c, :],
                                  KT[hsl, h * 128:(h + 1) * 128],
                                  QT[hsl, h * 128:(h + 1) * 128],
                                  start=True, stop=True)
                      eu = ebufp.tile([128, 8, 128], BF16, tag="ex",
                                      name="eu")
                      nc.scalar.activation(eu[:], sps2[:], Act.Exp,
                                           scale=SCALE)
                      return eu

                  def d_clip(eu):
                      nc.vector.tensor_scalar(eu[:], eu[:], EXP_LO, EXP_HI,
                                              Alu.max, Alu.min)

                  def d_values(hb, eu):
                      xups = psp.tile([128, 8, 128], F32, tag="P", bufs=3,
                                      name="xups")
                      for c in range(4):
                          h = hb * 4 + c
                          for jh in range(2):
                              j = jh * 4 + c
                              rhs = (xv2[jh][:, h, :] if KNORM
                                     else xv2[jh][:, :, h])
                              nc.tensor.matmul(
                                  xups[:, j, 0:D + 1],
                                  eu[:, j, :], rhs,
                                  start=True, stop=True)
                      ob = obufp.tile([128, 2, 4, D + 1], BF16, tag="ob",
                                      name="ob")
                      # (j c) d -> j c d is an identity layout map: use a
                      # flat 3D copy instead of a 4D AP
                      nc.vector.tensor_copy(
                          ob[:].rearrange("p j c d -> p (j c) d"),
                          xups[:, :, 0:D + 1])
                      # one DMA per hb covering both heads (1040B per
                      # partition), alternating queues for drain overlap
                      eng = nc.sync if hb % 2 == 0 else nc.gpsimd
                      eng.dma_start(out_d[g, hb], ob[:])

                  eus = {}
                  for hb in range(36):
                      if hb < 32:
                          eus[hb] = d_scores(hb)
                          if merge:
                              a_proj((g + 1) % NGROUPS, hb, VTh)
                      if hb - 4 >= 0:
                          d_values(hb - 4, eus[hb - 4])
                          del eus[hb - 4]
                      if 0 <= hb - 1 < 32:
                          d_clip(eus[hb - 1])

    nc.compile()
    return nc


def _get_nc():
    if "nc" not in _CACHE:
        _CACHE["nc"] = _build_bass()
    return _CACHE["nc"]


def kernel(x, wq, bq, wk, bk, wv, bv):
    from concourse.bass_utils import run_bass_kernel_spmd

    x = np.asarray(x, dtype=np.float32)
    wq = np.asarray(wq, dtype=np.float32)
    wk = np.asarray(wk, dtype=np.float32)
    wv = np.asarray(wv, dtype=np.float32)
    bq = np.asarray(bq, dtype=np.float32)
    bk = np.asarray(bk, dtype=np.float32)
    bv = np.asarray(bv, dtype=np.float32)

    nc = _get_nc()

    in_maps = []
    for core in range(NCORES):
        b = core // 2
        g2 = core % 2
        heads = list(range(g2 * 4, g2 * 4 + 4))
        cols = np.concatenate(
            [np.arange(n * D, (n + 1) * D) for n in heads])
        xb = x[b].reshape(PIX, C)
        xT = np.ascontiguousarray(xb.T).reshape(4, 128, PIX)
        in_maps.append({
            "xT": xT.astype(ml_dtypes.bfloat16),
            "wq": np.ascontiguousarray(wq[:, cols]).reshape(
                4, 128, 256).astype(ml_dtypes.bfloat16),
            "wk": np.ascontiguousarray(wk[:, cols]).reshape(
                4, 128, 256).astype(ml_dtypes.bfloat16),
            "wv": np.ascontiguousarray(wv[:, cols]).reshape(
                4, 128, 256).astype(ml_dtypes.bfloat16),
            "bq": np.ascontiguousarray(bq[cols].reshape(2, 128).T),
            "bk": np.ascontiguousarray(bk[cols].reshape(2, 128).T),
            "bv": np.ascontiguousarray(bv[cols].reshape(2, 128).T),
        })

    res = run_bass_kernel_spmd(nc, in_maps, list(range(NCORES)),
                               trace=bool(os.environ.get("KTRACE")))
    _CACHE["last_results"] = res

    out = np.empty((B, H, W, C), dtype=np.float32)
    for core in range(NCORES):
        r = np.asarray(res.results[core]["out"], dtype=np.float32)
        b = core // 2
        g2 = core % 2
        for jn, n in enumerate(range(g2 * 4, g2 * 4 + 4)):
            # r is [g, hb, w, jh, hr, d+1]; head jn = g*2 + jh. Take
            # [hb, w, hr, d+1] -> [h, w, d+1]; divide by the row-softmax
            # denominator (channel 64); reference channel order is d*NH+n
            a = r[jn // 2][:, :, jn % 2].transpose(
                0, 2, 1, 3).reshape(H, W, D + 1)
            out[b, :, :, n::NH] = a[:, :, 0:D] / a[:, :, D:D + 1]
    return out



# revision 47
# speedup vs baseline: 1.1125x; 1.1125x over previous
"""MultiHeadAxialAttention TRN2 kernel (~474us measured; prior 492us, stub 772us).

Problem: x[4,128,128,512] -> 1x1 conv q/k/v projections -> axial attention
(column attention over H, then row attention over W, per head) -> [4,128,128,512].

Sharding: core = (batch b, head-group of 4 heads); 8 cores, zero cross-core
communication. Host pre-transposes x[b] to x^T [512, 16384]; host reassembles
the per-core outputs into the reference channel order (channel = d*8 + n) and
applies the final row-softmax division (denominator is DMA'd as channel 64).

Key structure (softwarepipeline basics as before: lookahead-2 emission in both
attention passes, clips one iteration late, rotating psum tag pools, two heads
in lockstep on PE row-tiles T0/T8, double-stored K (h-major KT / w-major KTw),
next group's projections interleaved into the current group's row pass).

492us -> 474us changes (trace-driven):
  - column-softmax normalization moved from phase C into the phase-B psum
    evacuation (reciprocal of the ride-along sum plane + one fused
    multiply), so phase C is pure transposes + copies.
  - xv2 stored h-major [w, h, d]: phase D's value matmuls get a contiguous
    moving operand. Strided moving operands measured 113ns vs 54ns per MM
    (the same 2x stride penalty shows in B-scores: 218ns/pair vs D's 81ns).
  - phase C transposes batched 8 d-planes per psum tile; evacuation is an
    (h, d)-transposed AP split ACT (h 0:72) / DVE (h 72:128) - ACT handles
    strided writes at ~1.65ns/el vs DVE ~2ns/el.
  - proj psum evacuations rebalanced: contiguous q-evac on DVE, strided
    KTw/VTh evacs on ACT (the ACT chain is the psum-free critical path).
  - output DMAs split across sync + gpsimd queues (end-of-kernel drain);
    weight loads split sync/gpsimd; x prefetch on the scalar queue.

Measured dead ends (kept behind env flags, default off): HAM warm-keeper
matmuls (KWARM/KWARM2: the PE clock gate throttles to 1.2GHz during the
low-duty score/value phases and neither periodic nor burst fillers hold it
warm - the phases' intrinsic array duty is below the MID threshold), evac
halving across engines (KSPLIT: DVE is slow on strided writes), and
interleaving the A2 transposes into phase B (batched transposes pipeline
better than interleaved ones). Also measured TWICE: enlarging the output
DMAs to 1040B-per-partition contiguous runs (pairing two h-blocks, or
merging both heads into one DMA per hb with a head-pair-inner DRAM
layout) regressed 470->552-554us in BOTH variants - including one with
unchanged ob tile lifetimes - so the ~133KB merged transfer itself (not
tile holding) stalls the pipeline. The ~14-18us end-of-kernel drain tail
of 520B packets appears engine-pool rate-bound and is cheaper than any
fix tried. Run-to-run variance is ~+-8us (free-running HAM window).
"""
import sys
import os
import math

import numpy as np
import ml_dtypes

if "/opt/trn_rl_repo" not in sys.path:
    sys.path.insert(0, "/opt/trn_rl_repo")

B, H, W, C = 4, 128, 128, 512
NH, D = 8, 64
NCORES = 8
NGROUPS = 2          # head groups per core, 2 heads each
PIX = H * W          # 16384, h-major (pix = h*128 + w)
CLIP = 1.0 - 1e-7
SCALE = 1.0 / math.sqrt(D)   # 1/8
EXP_LO = float(np.float32(math.exp(-CLIP * SCALE)))
EXP_HI = float(np.float32(math.exp(CLIP * SCALE)))

_CACHE = {}


def _build_bass():
    import concourse.bacc as bacc
    import concourse.tile as tile
    import concourse.mybir as mybir
    from concourse import masks

    F32 = mybir.dt.float32
    BF16 = mybir.dt.bfloat16
    Act = mybir.ActivationFunctionType
    Alu = mybir.AluOpType

    nc = bacc.Bacc(None, target_bir_lowering=False)

    xT_d = nc.dram_tensor("xT", [4, 128, PIX], BF16, kind="ExternalInput")
    wq_d = nc.dram_tensor("wq", [4, 128, 256], BF16, kind="ExternalInput")
    wk_d = nc.dram_tensor("wk", [4, 128, 256], BF16, kind="ExternalInput")
    wv_d = nc.dram_tensor("wv", [4, 128, 256], BF16, kind="ExternalInput")
    bq_d = nc.dram_tensor("bq", [128, 2], F32, kind="ExternalInput")
    bk_d = nc.dram_tensor("bk", [128, 2], F32, kind="ExternalInput")
    bv_d = nc.dram_tensor("bv", [128, 2], F32, kind="ExternalInput")
    # [head, hb, w, h_in_block, d+1] — exactly the ob tile layout, so the
    # output DMA is one contiguous 66KB block (strided dst measured ~6GB/s)
    out_d = nc.dram_tensor("out", [4, H // 4, W, 4, D + 1], BF16,
                           kind="ExternalOutput")

    with tile.TileContext(nc) as tc:
        with (
            tc.tile_pool(name="const", bufs=1) as constp,
            tc.tile_pool(name="persist", bufs=1) as persist,
            tc.tile_pool(name="xt", bufs=4) as xtp,
            tc.tile_pool(name="ebuf", bufs=5) as ebufp,
            tc.tile_pool(name="rsbuf", bufs=2) as rsp,
            tc.tile_pool(name="obuf", bufs=4) as obufp,
            tc.tile_pool(name="ps", bufs=4, space="PSUM") as psp,
        ):
            ident_bf16 = constp.tile([128, 128], BF16, tag="id16")
            ident_f32 = constp.tile([128, 128], F32, tag="id32")
            masks.make_identity(nc, ident_bf16[:])
            masks.make_identity(nc, ident_f32[:])

            wsb = {}
            bsb = {}
            xt0 = None
            for nm, wd, bd in (("q", wq_d, bq_d), ("k", wk_d, bk_d),
                               ("v", wv_d, bv_d)):
                wt = constp.tile([128, 4, 256], BF16, tag=f"w{nm}")
                for kc in range(4):
                    # split weight loads across two queues at startup
                    eng = nc.sync if kc % 2 == 0 else nc.gpsimd
                    eng.dma_start(wt[:, kc, :], wd[kc])
                if xt0 is None:
                    # prefetch the first pixel tile right after the q
                    # weights so the first projection chain starts ~6us
                    # before the k/v weight DMAs finish
                    xt0 = xtp.tile([128, 4, 512], BF16, tag="xt", name="xt0")
                    nc.scalar.dma_start(
                        xt0[:], xT_d[:, :, 0:512].transpose([1, 0, 2]))
                bt = constp.tile([128, 2], F32, tag=f"b{nm}")
                nc.sync.dma_start(bt[:], bd[:])
                wsb[nm] = wt
                bsb[nm] = bt

            QT = persist.tile([128, PIX], BF16, tag="QT")
            KT = persist.tile([128, PIX], BF16, tag="KT")
            # second copy of K in w-major order: the column-pass score
            # matmuls get contiguous LDWEIGHTS (strided measured 2.3x slower)
            KTw = persist.tile([128, PIX], BF16, tag="KTw")

            NT = PIX // 512   # 32 pixel tiles of 512

            KREPS = int(os.environ.get("KREPS", "1"))
            KSPLIT = os.environ.get("KSPLIT", "0") != "0"
            KWARM = os.environ.get("KWARM", "0") != "0"
            KWARM2 = os.environ.get("KWARM2", "0") != "0"
            KNORM = os.environ.get("KNORM", "1") != "0"

            def warm_mm(ps_slice):
                # dense full-array N=512 matmul on const weights: keeps the
                # PE HAM activity monitor at K=8/8 through the low-duty
                # score/value phases (result is overwritten by real scores)
                nc.tensor.matmul(
                    ps_slice.rearrange("p a b -> p (a b)"),
                    wsb["q"][:, 0, 0:128],
                    wsb["k"][:, 0:2, :].rearrange("p a b -> p (a b)"),
                    start=True, stop=True)

            def strided_evac(dstv, srcv, bias):
                # [128, 128, 4] strided psum evacuation split across ACT
                # and DVE halves so the psum buffer frees ~2x sooner
                nc.scalar.activation(
                    dstv[:, 0:64, :], srcv[:, 0:64, :],
                    Act.Identity, bias=bias, scale=1.0)
                nc.vector.tensor_scalar(
                    dstv[:, 64:128, :], srcv[:, 64:128, :],
                    bias, None, Alu.add)

            def a_proj(g, tt, VTh, xt=None):
                fsl = slice(g * 128, (g + 1) * 128)
                if xt is None:
                    xt = xtp.tile([128, 4, 512], BF16, tag="xt", name="xt")
                    nc.sync.dma_start(
                        xt[:],
                        xT_d[:, :, tt * 512:(tt + 1) * 512].transpose(
                            [1, 0, 2]))
                for nm, dst in (("q", QT), ("k", KT), ("v", VTh)):
                    ps = psp.tile([128, 512], F32, tag="Q", bufs=2, name="ps")
                    for kc in range(4):
                        nc.tensor.matmul(
                            ps[:], wsb[nm][:, kc, fsl], xt[:, kc, :],
                            start=(kc == 0), stop=(kc == 3))
                    if nm == "k":
                        nc.vector.tensor_scalar(
                            dst[:, tt * 512:(tt + 1) * 512], ps[:],
                            bsb[nm][:, g:g + 1], None, Alu.add)
                        kv = KTw.rearrange("p (w h) -> p w h", w=128)[
                            :, :, tt * 4:(tt + 1) * 4]
                        src = ps[:].rearrange("p (h w) -> p w h", h=4)
                        if KSPLIT:
                            strided_evac(kv, src, bsb[nm][:, g:g + 1])
                        else:
                            nc.scalar.activation(
                                kv, src, Act.Identity,
                                bias=bsb[nm][:, g:g + 1], scale=1.0)
                    elif nm == "q":
                        # contiguous evac on DVE: frees the ACT queue for
                        # the strided KTw/VTh evacs (ACT is the proj-phase
                        # psum-free critical path)
                        nc.vector.tensor_scalar(
                            dst[:, tt * 512:(tt + 1) * 512], ps[:],
                            bsb[nm][:, g:g + 1], None, Alu.add)
                    else:
                        # VTh is w-major (pix2 = w*128 + h) so phase A2's
                        # transpose reads are contiguous
                        vv = VTh.rearrange("p (w h) -> p w h", w=128)[
                            :, :, tt * 4:(tt + 1) * 4]
                        src = ps[:].rearrange("p (h w) -> p w h", h=4)
                        if KSPLIT:
                            strided_evac(vv, src, bsb[nm][:, g:g + 1])
                        else:
                            nc.scalar.activation(
                                vv, src, Act.Identity,
                                bias=bsb[nm][:, g:g + 1], scale=1.0)

            for rep in range(KREPS):
              VTh = persist.tile([128, PIX], BF16, tag="big", name="VTh")
              for tt in range(NT):
                  a_proj(0, tt, VTh, xt=(xt0 if tt == 0 and rep == 0
                                         else None))
              for g in range(NGROUPS):
                  # ---- phase A2: V^T[f, (w h)] -> V_sb[h, w, d] ----
                  # emitted interleaved into phase B (below): the dense
                  # 128-row transposes raise B's PE duty (HAM stays warm)
                  # and fill B's dependency bubbles
                  V_sb = [persist.tile([128, W, D + 1], BF16, tag=f"Vx{j}",
                                       name=f"V{j}") for j in range(2)]
                  for jh in range(2):
                      nc.vector.memset(V_sb[jh][:, :, D], 1.0)

                  def a2_block(wb2):
                      tps = psp.tile([128, 8, 128], BF16, tag="Q", bufs=2,
                                     name="tps")
                      for j in range(8):
                          w = wb2 * 8 + j
                          nc.tensor.transpose(
                              tps[:, j, :], VTh[:, w * 128:(w + 1) * 128],
                              ident_bf16[:])
                      nc.scalar.copy(
                          V_sb[0][:, wb2 * 8:wb2 * 8 + 8, 0:D],
                          tps[:, :, 0:64])
                      nc.vector.tensor_copy(
                          V_sb[1][:, wb2 * 8:wb2 * 8 + 8, 0:D],
                          tps[:, :, 64:128])

                  # ---- phase B: column attention, heads paired ----
                  xv_sb = persist.tile([128, 2, D + 1, W], BF16, tag="big",
                                       name="xv_sb")

                  def b_scores(wb):
                      sps = psp.tile([128, 8, 128], F32, tag="P", bufs=3,
                                     name="sps")
                      if KWARM:
                          warm_mm(sps[:, 0:4, :])
                      if KWARM2:
                          # burst of dense N=512 matmuls at phase entry to
                          # flip the HAM clock gate to 8/8, then a hold
                          # warmer every other iteration
                          if wb == 0:
                              for _ in range(16):
                                  warm_mm(sps[:, 0:4, :])
                          elif wb % 2 == 0:
                              warm_mm(sps[:, 0:4, :])
                      for c in range(4):
                          w = wb * 4 + c
                          for jh in range(2):
                              hsl = slice(jh * 64, (jh + 1) * 64)
                              nc.tensor.matmul(
                                  sps[:, jh * 4 + c, :],
                                  KTw[hsl, w * 128:(w + 1) * 128],
                                  QT[hsl, w::128],
                                  start=True, stop=True)
                      ex = ebufp.tile([128, 8, 128], BF16, tag="ex",
                                      name="ex")
                      nc.scalar.activation(ex[:], sps[:], Act.Exp,
                                           scale=SCALE)
                      return ex

                  def b_clip(ex):
                      nc.vector.tensor_scalar(ex[:], ex[:], EXP_LO, EXP_HI,
                                              Alu.max, Alu.min)

                  def b_values(wb, ex):
                      xvps = psp.tile([128, 8, 128], F32, tag="P", bufs=3,
                                      name="xvps")
                      for c in range(4):
                          w = wb * 4 + c
                          for jh in range(2):
                              j = jh * 4 + c
                              nc.tensor.matmul(
                                  xvps[:, j, 0:D + 1],
                                  ex[:, j, :], V_sb[jh][:, w, :],
                                  start=True, stop=True)
                      if KNORM:
                          # fold the column-softmax normalization into the
                          # evacuation: xv_sb holds xv/sv (plane D becomes 1)
                          rs = rsp.tile([128, 8, 1], F32, tag="rsb2",
                                        bufs=2, name="rs")
                          nc.vector.reciprocal(rs[:], xvps[:, :, D:D + 1])
                          nc.vector.tensor_tensor(
                              xv_sb[:, :, :, wb * 4:wb * 4 + 4],
                              xvps[:, :, 0:D + 1].rearrange(
                                  "p (j c) d -> p j d c", j=2),
                              rs[:].rearrange(
                                  "p (j c) u -> p j u c", j=2).broadcast_to(
                                  [128, 2, D + 1, 4]),
                              Alu.mult)
                      else:
                          nc.vector.tensor_copy(
                              xv_sb[:, :, :, wb * 4:wb * 4 + 4],
                              xvps[:, :, 0:D + 1].rearrange(
                                  "p (j c) d -> p j d c", j=2))

                  for wb2 in range(16):
                      a2_block(wb2)

                  exs = {}
                  for wb in range(36):
                      if wb < 32:
                          exs[wb] = b_scores(wb)
                      if wb - 4 >= 0:
                          b_values(wb - 4, exs[wb - 4])
                          del exs[wb - 4]
                      if 0 <= wb - 1 < 32:
                          b_clip(exs[wb - 1])

                  # ---- phase C: xv [h, jh, d, w] -> xv2 ----
                  if KNORM:
                      # xv already normalized at the B evac; xv2 stored
                      # h-major [w, h, d] so phase D's value matmuls get a
                      # contiguous moving operand (strided moving measured
                      # 113ns vs 54ns per MM)
                      xv2 = [persist.tile([128, H, D + 1], BF16,
                                          tag=f"Vx{j}", name=f"xv2_{j}")
                             for j in range(2)]
                      for jh in range(2):
                          for db in range(8):
                              # 8 d-planes per psum tile; evacuate with an
                              # (h, d)-transposed AP split across ACT and
                              # DVE h-ranges (ACT takes more: it handles
                              # strided writes at ~1.65ns/el vs DVE ~2)
                              mps = psp.tile([128, 8, 128], BF16, tag="Q",
                                             bufs=2, name="mps")
                              for j in range(8):
                                  d = db * 8 + j
                                  nc.tensor.transpose(
                                      mps[:, j, :], xv_sb[:, jh, d, :],
                                      ident_bf16[:])
                              src = mps[:].rearrange("p j h -> p h j")
                              dst = xv2[jh][:, :, db * 8:db * 8 + 8]
                              nc.scalar.copy(dst[:, 0:72, :], src[:, 0:72, :])
                              nc.vector.tensor_copy(dst[:, 72:128, :],
                                                    src[:, 72:128, :])
                          nc.vector.memset(xv2[jh][:, :, D], 1.0)
                  else:
                      xv2 = [persist.tile([128, D + 1, H], BF16,
                                          tag=f"Vx{j}", name=f"xv2_{j}")
                             for j in range(2)]
                      for jh in range(2):
                          stp = psp.tile([128, 128], BF16, tag="Q", bufs=2,
                                         name="stp")
                          nc.tensor.transpose(stp[:], xv_sb[:, jh, D, :],
                                              ident_bf16[:])
                          rsvT_f = rsp.tile([128, H], F32, tag="rsf",
                                            name="rsvT_f")
                          nc.vector.reciprocal(rsvT_f[:], stp[:])
                          rsvT = rsp.tile([128, H], BF16, tag="rsb",
                                          name="rsvT")
                          nc.vector.tensor_copy(rsvT[:], rsvT_f[:])
                          for db in range(16):
                              mps = psp.tile([128, 4, 128], BF16, tag="Q",
                                             bufs=2, name="mps")
                              for j in range(4):
                                  d = db * 4 + j
                                  nc.tensor.transpose(
                                      mps[:, j, :], xv_sb[:, jh, d, :],
                                      ident_bf16[:])
                              nc.vector.tensor_tensor(
                                  xv2[jh][:, db * 4:db * 4 + 4, :], mps[:],
                                  rsvT[:].unsqueeze(1).broadcast_to(
                                      [128, 4, 128]),
                                  Alu.mult)
                          nc.vector.memset(xv2[jh][:, D, :], 1.0)

                  # ---- phase D: row attention; next group's projections
                  # are interleaved per-hb (QT/KT slice tt is last read by
                  # the scores of hb=tt, so the overwrite pipelines) ----
                  merge = (g + 1 < NGROUPS) or (rep + 1 < KREPS)
                  if merge:
                      VTh = persist.tile([128, PIX], BF16, tag="big",
                                         name="VTh")

                  def d_scores(hb):
                      sps2 = psp.tile([128, 8, 128], F32, tag="P", bufs=3,
                                      name="sps2")
                      if KWARM and not merge:
                          warm_mm(sps2[:, 0:4, :])
                      if KWARM2 and not merge:
                          if hb == 0:
                              for _ in range(16):
                                  warm_mm(sps2[:, 0:4, :])
                          elif hb % 2 == 0:
                              warm_mm(sps2[:, 0:4, :])
                      for c in range(4):
                          h = hb * 4 + c
                          for jh in range(2):
                              hsl = slice(jh * 64, (jh + 1) * 64)
                              nc.tensor.matmul(
                                  sps2[:, jh * 4 + c, :],
                                  KT[hsl, h * 128:(h + 1) * 128],
                                  QT[hsl, h * 128:(h + 1) * 128],
                                  start=True, stop=True)
                      eu = ebufp.tile([128, 8, 128], BF16, tag="ex",
                                      name="eu")
                      nc.scalar.activation(eu[:], sps2[:], Act.Exp,
                                           scale=SCALE)
                      return eu

                  def d_clip(eu):
                      nc.vector.tensor_scalar(eu[:], eu[:], EXP_LO, EXP_HI,
                                              Alu.max, Alu.min)

                  def d_values(hb, eu):
                      xups = psp.tile([128, 8, 128], F32, tag="P", bufs=3,
                                      name="xups")
                      for c in range(4):
                          h = hb * 4 + c
                          for jh in range(2):
                              j = jh * 4 + c
                              rhs = (xv2[jh][:, h, :] if KNORM
                                     else xv2[jh][:, :, h])
                              nc.tensor.matmul(
                                  xups[:, j, 0:D + 1],
                                  eu[:, j, :], rhs,
                                  start=True, stop=True)
                      ob = obufp.tile([128, 2, 4, D + 1], BF16, tag="ob",
                                      name="ob")
                      # (j c) d -> j c d is an identity layout map: use a
                      # flat 3D copy instead of a 4D AP
                      nc.vector.tensor_copy(
                          ob[:].rearrange("p j c d -> p (j c) d"),
                          xups[:, :, 0:D + 1])
                      if merge:
                          # two DMA queues (sync also carries the xt
                          # prefetches; scalar carries proj evac work)
                          nc.sync.dma_start(out_d[g * 2, hb], ob[:, 0])
                          nc.gpsimd.dma_start(out_d[g * 2 + 1, hb],
                                              ob[:, 1])
                      else:
                          # last group: no xt/proj DMA traffic, so round-
                          # robin all three DMA-capable queues — the drain
                          # is packet-rate bound (~18us tail on 2 queues)
                          engs = [nc.sync, nc.gpsimd, nc.scalar]
                          engs[(2 * hb) % 3].dma_start(
                              out_d[g * 2, hb], ob[:, 0])
                          engs[(2 * hb + 1) % 3].dma_start(
                              out_d[g * 2 + 1, hb], ob[:, 1])

                  eus = {}
                  for hb in range(36):
                      if hb < 32:
                          eus[hb] = d_scores(hb)
                          if merge:
                              a_proj((g + 1) % NGROUPS, hb, VTh)
                      if hb - 4 >= 0:
                          d_values(hb - 4, eus[hb - 4])
                          del eus[hb - 4]
                      if 0 <= hb - 1 < 32:
                          d_clip(eus[hb - 1])

    nc.compile()
    return nc


def _get_nc():
    if "nc" not in _CACHE:
        _CACHE["nc"] = _build_bass()
    return _CACHE["nc"]


def kernel(x, wq, bq, wk, bk, wv, bv):
    from concourse.bass_utils import run_bass_kernel_spmd

    x = np.asarray(x, dtype=np.float32)
    wq = np.asarray(wq, dtype=np.float32)
    wk = np.asarray(wk, dtype=np.float32)
    wv = np.asarray(wv, dtype=np.float32)
    bq = np.asarray(bq, dtype=np.float32)
    bk = np.asarray(bk, dtype=np.float32)
    bv = np.asarray(bv, dtype=np.float32)

    nc = _get_nc()

    in_maps = []
    for core in range(NCORES):
        b = core // 2
        g2 = core % 2
        heads = list(range(g2 * 4, g2 * 4 + 4))
        cols = np.concatenate(
            [np.arange(n * D, (n + 1) * D) for n in heads])
        xb = x[b].reshape(PIX, C)
        xT = np.ascontiguousarray(xb.T).reshape(4, 128, PIX)
        in_maps.append({
            "xT": xT.astype(ml_dtypes.bfloat16),
            "wq": np.ascontiguousarray(wq[:, cols]).reshape(
                4, 128, 256).astype(ml_dtypes.bfloat16),
            "wk": np.ascontiguousarray(wk[:, cols]).reshape(
                4, 128, 256).astype(ml_dtypes.bfloat16),
            "wv": np.ascontiguousarray(wv[:, cols]).reshape(
                4, 128, 256).astype(ml_dtypes.bfloat16),
            "bq": np.ascontiguousarray(bq[cols].reshape(2, 128).T),
            "bk": np.ascontiguousarray(bk[cols].reshape(2, 128).T),
            "bv": np.ascontiguousarray(bv[cols].reshape(2, 128).T),
        })

    res = run_bass_kernel_spmd(nc, in_maps, list(range(NCORES)),
                               trace=bool(os.environ.get("KTRACE")))
    _CACHE["last_results"] = res

    out = np.empty((B, H, W, C), dtype=np.float32)
    for core in range(NCORES):
        r = np.asarray(res.results[core]["out"], dtype=np.float32)
        b = core // 2
        g2 = core % 2
        for jn, n in enumerate(range(g2 * 4, g2 * 4 + 4)):
            # r[jn] is [hb, w, hr, d+1] -> [h, w, d+1]; divide by the
            # row-softmax denominator (channel 64); reference channel
            # order is d*NH + n
            a = r[jn].transpose(0, 2, 1, 3).reshape(H, W, D + 1)
            out[b, :, :, n::NH] = a[:, :, 0:D] / a[:, :, D:D + 1]
    return out

